# revision 17
# baseline (speedup 1.0000x reference)
"""Trainium2 Bass kernel for a dense transformer block.

Shapes: B=512, T=256, C=512, H=4 (HS=128), FF=2048, NF=2 dual-FFN.
Sharding: data-parallel over batch across 8 NeuronCores (64 batches/core),
all parameters replicated, no collectives.

On-chip layout is fully "transposed": activations live as
[feature -> partitions (chunks of 128), tokens -> free], so every matmul
contracts over the partition axis with no on-chip transposes. The host
pre-transposes x to [B, C, T] and pre-arranges weights into SBUF-friendly
layouts (cast to bf16). Matmul accumulation is fp32 in PSUM.
"""

import numpy as np
import ml_dtypes

import concourse.bass as bass
import concourse.bacc as bacc
import concourse.tile as tile
from concourse import mybir
from concourse.bass_utils import run_bass_kernel_spmd

F32 = mybir.dt.float32
F32R = mybir.dt.float32r
BF16 = mybir.dt.bfloat16
AF = mybir.ActivationFunctionType
ALU = mybir.AluOpType

B, T, C, H = 512, 256, 512, 4
HS = C // H            # 128
FF = 4 * C             # 2048
NF = 2
EPS = 1e-8
N_CORES = 8
NB = B // N_CORES      # 64 batches per core
P = 128
CC = C // P            # 4 c-chunks
FC = FF // P           # 16 f-chunks
T2 = 2 * T             # tokens per 2-batch group
SCALE = C ** -0.5


def _build(nb: int):
    """Build the Bass program for nb batches (nb even)."""
    nc = bacc.Bacc("TRN2", target_bir_lowering=False, debug=False,
                   num_devices=N_CORES)

    xT = nc.dram_tensor("xT", [nb, C, T], F32, kind="ExternalInput")
    wq = nc.dram_tensor("wq", [P, CC, H, HS], BF16, kind="ExternalInput")
    wk = nc.dram_tensor("wk", [P, CC, H, HS], BF16, kind="ExternalInput")
    wv = nc.dram_tensor("wv", [P, CC, H, HS], BF16, kind="ExternalInput")
    bk = nc.dram_tensor("bk", [1, H, HS], BF16, kind="ExternalInput")
    wproj = nc.dram_tensor("wproj", [P, CC, C], BF16, kind="ExternalInput")
    bproj = nc.dram_tensor("bproj", [P, CC], F32, kind="ExternalInput")
    w1 = nc.dram_tensor("w1", [1, 1], F32, kind="ExternalInput")
    w2 = nc.dram_tensor("w2", [1, 1], F32, kind="ExternalInput")
    wi = nc.dram_tensor("wi", [P, NF, CC, FF], BF16, kind="ExternalInput")
    wo = nc.dram_tensor("wo", [P, NF, FC, C], BF16, kind="ExternalInput")
    triu = nc.dram_tensor("triu", [P, P], BF16, kind="ExternalInput")
    outT = nc.dram_tensor("outT", [nb, C, T], F32, kind="ExternalOutput")

    from contextlib import ExitStack
    with tile.TileContext(nc) as tc, ExitStack() as ctx:
        wp = ctx.enter_context(tc.tile_pool(name="wp", bufs=1))
        sp = ctx.enter_context(tc.tile_pool(name="sp", bufs=2))
        ep = ctx.enter_context(tc.tile_pool(name="ep", bufs=3))
        hp = ctx.enter_context(tc.tile_pool(name="hp", bufs=3))
        ps = ctx.enter_context(tc.tile_pool(name="ps", bufs=8, space="PSUM"))

        # ---- persistent weights / constants ----
        wq_sb = wp.tile([P, CC, H, HS], BF16)
        nc.sync.dma_start(out=wq_sb, in_=wq[:])
        wk_sb = wp.tile([P, CC, H, HS], BF16)
        nc.sync.dma_start(out=wk_sb, in_=wk[:])
        wv_sb = wp.tile([P, CC, H, HS], BF16)
        nc.sync.dma_start(out=wv_sb, in_=wv[:])
        bk_sb = wp.tile([1, H, HS], BF16)
        nc.sync.dma_start(out=bk_sb, in_=bk[:])
        wproj_sb = wp.tile([P, CC, C], BF16)
        nc.sync.dma_start(out=wproj_sb, in_=wproj[:])
        bproj_sb = wp.tile([P, CC], F32)
        nc.sync.dma_start(out=bproj_sb, in_=bproj[:])
        w1_sb = wp.tile([1, 1], F32)
        nc.sync.dma_start(out=w1_sb, in_=w1[:])
        w2_sb = wp.tile([1, 1], F32)
        nc.sync.dma_start(out=w2_sb, in_=w2[:])
        wi_sb = wp.tile([P, NF, CC, FF], BF16)
        nc.sync.dma_start(out=wi_sb, in_=wi[:])
        wo_sb = wp.tile([P, NF, FC, C], BF16)
        nc.sync.dma_start(out=wo_sb, in_=wo[:])
        triu_sb = wp.tile([P, P], BF16)
        nc.sync.dma_start(out=triu_sb, in_=triu[:])

        ones_col = wp.tile([P, 1], BF16)     # lhsT for partition sums (bf16)
        nc.vector.memset(ones_col, 1.0)
        ones_f32 = wp.tile([P, 1], F32)
        nc.vector.memset(ones_f32, 1.0)
        ones_col_f = wp.tile([P, 1], F32R)
        nc.vector.tensor_copy(ones_col_f, ones_f32)
        ones_row_f = wp.tile([1, P], F32R)
        ones_row_f32 = wp.tile([1, P], F32)
        nc.vector.memset(ones_row_f32, 1.0)
        nc.vector.tensor_copy(ones_row_f, ones_row_f32)
        ones_t = wp.tile([1, T2], BF16)      # rhs for k-bias (K=1)
        nc.vector.memset(ones_t, 1.0)
        eps_sb = wp.tile([1, 1], F32)
        nc.vector.memset(eps_sb, EPS)

        def rmsnorm(src, w_sb, out_f32, out_bf16):
            """src [P, CC, 2, T] f32 -> out = w * src * rsqrt(mean_c(src^2))."""
            ssq = ps.tile([1, T2], F32, tag="ps")
            for cc in range(CC):
                sq = ep.tile([P, T2], F32R, tag="sq")
                nc.vector.tensor_mul(
                    sq, src[:, cc].rearrange("p a b -> p (a b)"),
                    src[:, cc].rearrange("p a b -> p (a b)"))
                nc.tensor.matmul(ssq, ones_col_f, sq,
                                 start=(cc == 0), stop=(cc == CC - 1))
            rms = ep.tile([1, T2], F32, tag="rms")
            nc.scalar.activation(out=rms, in_=ssq, func=AF.Sqrt,
                                 bias=eps_sb, scale=1.0 / C)
            rstd = ep.tile([1, T2], F32, tag="rstd")
            nc.vector.reciprocal(rstd, rms)
            g_row = ep.tile([1, T2], F32R, tag="g_row")
            nc.vector.tensor_scalar_mul(g_row, rstd, w_sb[0:1, 0:1])
            gb = ps.tile([P, T2], F32, tag="ps")
            nc.tensor.matmul(gb, ones_row_f, g_row, start=True, stop=True)
            for cc in range(CC):
                if out_f32 is not None:
                    nc.vector.tensor_mul(
                        out_f32[:, cc].rearrange("p a b -> p (a b)"),
                        src[:, cc].rearrange("p a b -> p (a b)"), gb)
                if out_bf16 is not None:
                    nc.vector.tensor_mul(
                        out_bf16[:, cc],
                        src[:, cc].rearrange("p a b -> p (a b)"), gb)

        for g in range(nb // 2):
            # ---- load x group: [P, CC, b2, T] ----
            xt = sp.tile([P, CC, 2, T], F32, tag="xt")
            for b2 in range(2):
                nc.sync.dma_start(
                    out=xt[:, :, b2],
                    in_=xT[2 * g + b2].rearrange("(cc p) t -> p cc t", p=P))

            ht = sp.tile([P, CC, T2], BF16, tag="ht")
            rmsnorm(xt, w1_sb, None, ht)

            # ---- q, k (transposed per head: [d, t2]), v ([s, (h d)]) ----
            qT = sp.tile([P, H, T2], BF16, tag="qT")
            kT = sp.tile([P, H, T2], BF16, tag="kT")
            for h in range(H):
                q_ps = ps.tile([P, T2], F32, tag="ps")
                for cc in range(CC):
                    nc.tensor.matmul(q_ps, wq_sb[:, cc, h], ht[:, cc],
                                     start=(cc == 0), stop=(cc == CC - 1))
                nc.scalar.copy(out=qT[:, h], in_=q_ps)
                k_ps = ps.tile([P, T2], F32, tag="ps")
                for cc in range(CC):
                    nc.tensor.matmul(k_ps, wk_sb[:, cc, h], ht[:, cc],
                                     start=(cc == 0), stop=False)
                nc.tensor.matmul(k_ps, bk_sb[:, h], ones_t,
                                 start=False, stop=True)
                nc.scalar.copy(out=kT[:, h], in_=k_ps)
            v_sb = sp.tile([P, 2, 2, H * HS], BF16, tag="v_sb")  # [s-chunk]
            for b2 in range(2):
                for th in range(2):
                    v_ps = ps.tile([P, H * HS], F32, tag="ps")
                    for cc in range(CC):
                        nc.tensor.matmul(
                            v_ps,
                            ht[:, cc, b2 * T + th * P:b2 * T + (th + 1) * P],
                            wv_sb[:, cc].rearrange("p a b -> p (a b)"),
                            start=(cc == 0), stop=(cc == CC - 1))
                    nc.scalar.copy(out=v_sb[:, b2, th], in_=v_ps)

            # ---- attention per (batch, head) ----
            attnT = sp.tile([P, H, 2, T], BF16, tag="attnT")
            for b2 in range(2):
                for h in range(H):
                    tsl = slice(b2 * T, (b2 + 1) * T)
                    # scores^T chunks: st0 [s0,(0:256)], st1 [s1,(128:256)]
                    st0 = ps.tile([P, T], F32, tag="ps")
                    nc.tensor.matmul(st0, kT[:, h, b2 * T:b2 * T + P],
                                     qT[:, h, tsl], start=True, stop=True)
                    st1 = ps.tile([P, P], F32, tag="ps")
                    nc.tensor.matmul(
                        st1, kT[:, h, b2 * T + P:b2 * T + T],
                        qT[:, h, b2 * T + P:b2 * T + T],
                        start=True, stop=True)
                    e0 = ep.tile([P, T], BF16, tag="e0")
                    nc.scalar.activation(out=e0, in_=st0, func=AF.Exp,
                                         scale=SCALE)
                    e1 = ep.tile([P, T], BF16, tag="e1")
                    nc.vector.memset(e1[:, 0:P], 0.0)
                    nc.scalar.activation(out=e1[:, P:T], in_=st1, func=AF.Exp,
                                         scale=SCALE)
                    # causal mask on the two diagonal blocks
                    nc.vector.tensor_mul(e0[:, 0:P], e0[:, 0:P], triu_sb)
                    nc.vector.tensor_mul(e1[:, P:T], e1[:, P:T], triu_sb)
                    den = ps.tile([1, T], F32, tag="ps")
                    nc.tensor.matmul(den, ones_col, e0, start=True, stop=False)
                    nc.tensor.matmul(den, ones_col, e1, start=False, stop=True)
                    rec = ep.tile([1, T], F32R, tag="rec")
                    with nc.allow_low_precision(reason="f32r softmax recip"):
                        nc.vector.reciprocal(rec, den)
                    bc = ps.tile([P, T], F32, tag="ps")
                    nc.tensor.matmul(bc, ones_row_f, rec,
                                     start=True, stop=True)
                    p0 = ep.tile([P, T], BF16, tag="p0")
                    nc.vector.tensor_mul(p0, e0, bc)
                    p1 = ep.tile([P, T], BF16, tag="p1")
                    nc.vector.tensor_mul(p1, e1, bc)
                    at_ps = ps.tile([P, T], F32, tag="ps")
                    nc.tensor.matmul(at_ps, v_sb[:, b2, 0, h * HS:(h + 1) * HS],
                                     p0, start=True, stop=False)
                    nc.tensor.matmul(at_ps, v_sb[:, b2, 1, h * HS:(h + 1) * HS],
                                     p1, start=False, stop=True)
                    nc.scalar.copy(out=attnT[:, h, b2], in_=at_ps)

            # ---- proj + bias + residual (in-place into xt) ----
            for cc in range(CC):
                y_ps = ps.tile([P, T2], F32, tag="ps")
                for h in range(H):
                    nc.tensor.matmul(y_ps, wproj_sb[:, h, cc * P:(cc + 1) * P],
                                     attnT[:, h].rearrange("p a b -> p (a b)"),
                                     start=(h == 0), stop=(h == H - 1))
                nc.vector.scalar_tensor_tensor(
                    out=xt[:, cc].rearrange("p a b -> p (a b)"),
                    in0=y_ps, scalar=bproj_sb[:, cc:cc + 1],
                    in1=xt[:, cc].rearrange("p a b -> p (a b)"),
                    op0=ALU.add, op1=ALU.add)

            # rmsnorm2: x2 (f32) staged directly in the output tile `ot`
            ot = sp.tile([P, CC, 2, T], F32, tag="ot")
            h2 = sp.tile([P, CC, T2], BF16, tag="h2")
            rmsnorm(xt, w2_sb, ot, h2)

            # ---- dual-branch FFN, mean of branches, + residual ----
            ff_ps = [ps.tile([P, T2], F32, tag="ps", name=f"ff{cc}")
                     for cc in range(CC)]
            for br in range(NF):
                for fc in range(FC):
                    hm_ps = ps.tile([P, T2], F32, tag="ps")
                    for cc in range(CC):
                        nc.tensor.matmul(
                            hm_ps, wi_sb[:, br, cc, fc * P:(fc + 1) * P],
                            h2[:, cc], start=(cc == 0), stop=(cc == CC - 1))
                    hm = hp.tile([P, T2], BF16, tag="hm")
                    nc.scalar.activation(out=hm, in_=hm_ps, func=AF.Gelu)
                    for cc in range(CC):
                        nc.tensor.matmul(
                            ff_ps[cc], wo_sb[:, br, fc, cc * P:(cc + 1) * P],
                            hm, start=(br == 0 and fc == 0),
                            stop=(br == NF - 1 and fc == FC - 1))

            for cc in range(CC):
                nc.vector.scalar_tensor_tensor(
                    out=ot[:, cc].rearrange("p a b -> p (a b)"),
                    in0=ff_ps[cc], scalar=0.5,
                    in1=ot[:, cc].rearrange("p a b -> p (a b)"),
                    op0=ALU.mult, op1=ALU.add)
            for b2 in range(2):
                nc.sync.dma_start(
                    out=outT[2 * g + b2].rearrange("(cc p) t -> p cc t", p=P),
                    in_=ot[:, :, b2])

    nc.finalize()
    return nc


def _prep_weights(Wq, Wk, bk, Wv, Wproj, bproj, w1, w2, Wi, Wo):
    bf = ml_dtypes.bfloat16
    def qkvw(w):  # [H, C, HS] -> [P, CC, H, HS]
        return np.ascontiguousarray(
            w.reshape(H, CC, P, HS).transpose(2, 1, 0, 3)).astype(bf)
    m = {
        "wq": qkvw(Wq), "wk": qkvw(Wk), "wv": qkvw(Wv),
        "bk": np.ascontiguousarray(bk.reshape(1, H, HS)).astype(bf),
        "wproj": np.ascontiguousarray(
            Wproj.reshape(CC, P, C).transpose(1, 0, 2)).astype(bf),
        "bproj": np.ascontiguousarray(
            bproj.reshape(CC, P).transpose(1, 0)).astype(np.float32),
        "w1": w1.reshape(1, 1).astype(np.float32),
        "w2": w2.reshape(1, 1).astype(np.float32),
        "wi": np.ascontiguousarray(
            Wi.reshape(NF, CC, P, FF).transpose(2, 0, 1, 3)).astype(bf),
        "wo": np.ascontiguousarray(
            Wo.reshape(NF, FC, P, C).transpose(2, 0, 1, 3)).astype(bf),
        "triu": np.triu(np.ones((P, P))).astype(bf),
    }
    return m


def kernel(x, Wq, Wk, bk, Wv, Wproj, bproj, w1, w2, Wi, Wo):
    x = np.asarray(x, np.float32)
    nb = x.shape[0] // N_CORES
    nc = _build(nb)
    wmap = _prep_weights(np.asarray(Wq), np.asarray(Wk), np.asarray(bk),
                         np.asarray(Wv), np.asarray(Wproj), np.asarray(bproj),
                         np.asarray(w1), np.asarray(w2), np.asarray(Wi),
                         np.asarray(Wo))
    in_maps = []
    for c in range(N_CORES):
        shard = x[c * nb:(c + 1) * nb]                      # [nb, T, C]
        xTc = np.ascontiguousarray(shard.transpose(0, 2, 1))  # [nb, C, T]
        in_maps.append({"xT": xTc, **wmap})
    res = run_bass_kernel_spmd(nc, in_maps, list(range(N_CORES)))
    outs = []
    for c in range(N_CORES):
        oT = res.results[c]["outT"]                         # [nb, C, T]
        outs.append(oT.transpose(0, 2, 1))                  # [nb, T, C]
    return np.ascontiguousarray(np.concatenate(outs, axis=0), np.float32)
